# revision 1
# baseline (speedup 1.0000x reference)
"""
MoE-routing kernel for Trainium2 (8 NeuronCores, SPMD via bass).

Computation (matches the reference):
  attended[b, c] = sum_hw((mask[b, hw] + 1e-10) * feat[b, c, hw]) / sum_hw(mask[b, hw] + 1e-10)
  out[b, a]      = attended[b, :] @ W[inst[b], a, :] + bias[inst[b], a]

Strategy: split the channel dim C=2048 into 8 shards of 256 (one per core).
Each core computes a partial contraction over its channel shard for ALL 256
samples; the host sums the 8 partials.  The batch is sorted by expert on the
host (static routing baked into the compiled program), so each expert's
samples form a contiguous group of stationary columns for the grouped GEMM.

Per core:
  phase 1 (pooling): for each sample, PE broadcasts the mask row to 128
    partitions (K=1 matmul), DVE does a fused multiply+reduce
    (tensor_tensor_reduce) against the feature tile -> one column of
    attended^T per (sample, c-tile).  Unnormalized (raw mask).
  phase 2 (grouped GEMM): per expert group, stationary = attended^T columns
    of the group, moving = W^T [c, a] chunks streamed from HBM (float32r ->
    full PE rate).  An extra K=1 matmul accumulates msum[b] * bias[e, a]
    into PSUM; eviction multiplies rows by 1/msum[b] (per-partition scalar),
    which normalizes the pooled features and leaves bias intact.
"""

import sys

if "/opt/trn_rl_repo" not in sys.path:
    sys.path.insert(0, "/opt/trn_rl_repo")

import numpy as np

import concourse.bass as bass
import concourse.mybir as mybir
import concourse.tile as tile
from concourse import bacc
from concourse import bass_utils
from concourse.masks import make_identity

# Problem constants (hardcoded; kernel.py must be self-contained)
B = 256          # batch
C = 2048         # channels
HW = 196         # spatial positions (14*14)
E = 16           # experts
A = 3000         # answers
NCORES = 8
CS = C // NCORES  # channel shard per core = 256
P = 128
KT = CS // P      # k-tiles per core = 2
MROW_BATCH = 16   # samples per partition-0 mask-row tile
HWP = 256         # padded mask row width (f32r wants moving free >= 256)
CHUNKS = [(c0, min(512, A - c0)) for c0 in range(0, A, 512)]

F32 = mybir.dt.float32
F32R = mybir.dt.float32r


def _make_groups(counts):
    """[(gstart_in_sorted_order, gsz, expert)] with gsz <= 128."""
    groups = []
    start = 0
    for e in range(E):
        n = int(counts[e])
        g0 = start
        while n > 0:
            gsz = min(n, P)
            groups.append((g0, gsz, e))
            g0 += gsz
            n -= gsz
        start += int(counts[e])
    return groups


def build_program(groups, loop_n=1, do_pool=True, do_mm=True, do_evict=True, pool_mode='full'):
    """Build + compile the per-core Bass program (identical on all cores)."""
    nc = bacc.Bacc("TRN2", target_bir_lowering=False, debug=False,
                   num_devices=NCORES)

    feat_d = nc.dram_tensor("feat", [B, CS, HW], F32, kind="ExternalInput").ap()
    mask_d = nc.dram_tensor("mask", [B, HW], F32, kind="ExternalInput").ap()
    wt_d = nc.dram_tensor("wt", [E, CS, A], F32R, kind="ExternalInput").ap()
    bias_d = nc.dram_tensor("bias", [1, E * A], F32R, kind="ExternalInput").ap()
    part_d = nc.dram_tensor("part", [B, A], F32, kind="ExternalOutput").ap()

    import contextlib
    with tile.TileContext(nc) as tc:
        loop_ctx = tc.For_i(0, loop_n, 1) if loop_n > 1 else contextlib.nullcontext()
        with (
            loop_ctx,
            tc.tile_pool(name="persist", bufs=1) as pp,
            tc.tile_pool(name="feat", bufs=6) as fp,
            tc.tile_pool(name="mrow", bufs=3) as mrp,
            tc.tile_pool(name="wt", bufs=6) as wtp,
            tc.tile_pool(name="bias", bufs=2) as bp,
            tc.tile_pool(name="outs", bufs=4) as op,
            tc.tile_pool(name="bcast", bufs=4) as pbc,
            tc.tile_pool(name="ps_mm", bufs=3, space="PSUM") as pmm,
            tc.tile_pool(name="ps_sm", bufs=2, space="PSUM") as psm,
        ):
            # ---- constants ----
            ident = pp.tile([P, P], F32, tag="ident")
            make_identity(nc, ident)
            ones32 = pp.tile([1, 1], F32, tag="ones32")
            nc.vector.memset(ones32, 1.0)
            dummy = pp.tile([P, 1], F32, tag="dummy")

            # ---- mask: per-sample sums and reciprocals ----
            mbp = pp.tile([P, KT, HW], F32, tag="mbp")
            nc.sync.dma_start(mbp, mask_d.rearrange("(t p) f -> p t f", p=P))
            msum = pp.tile([P, KT], F32, tag="msum")
            nc.vector.tensor_reduce(msum, mbp,
                                    axis=mybir.AxisListType.X,
                                    op=mybir.AluOpType.add)
            nc.vector.tensor_scalar_add(msum, msum, HW * 1e-10)

            # msum as a partition-0 row [1, B] (exact fp32 extraction matmuls)
            msum_row = pp.tile([1, B], F32R, tag="msum_row")
            for t in range(KT):
                pt = psm.tile([1, P], F32, name="pt_row", tag="pt")
                nc.tensor.matmul(pt, lhsT=msum[:, t:t + 1], rhs=ident,
                                 start=True, stop=True)
                nc.vector.tensor_copy(msum_row[0:1, t * P:(t + 1) * P], pt)
            recip_row = pp.tile([1, B], F32, tag="recip_row")
            nc.vector.reciprocal(recip_row, msum_row)

            # per-group reciprocals at partition base 0: rg[r] = 1/msum[g0+r]
            rgrps = []
            for gi, (g0, gsz, e) in enumerate(groups):
                rg = pp.tile([P, 1], F32, tag=f"rgrp{gi}", name=f"rgrp{gi}")
                pt = psm.tile([P, 1], F32, name="pt_col", tag="pt")
                nc.tensor.matmul(pt[:gsz], lhsT=recip_row[0:1, g0:g0 + gsz],
                                 rhs=ones32[0:1, 0:1], start=True, stop=True)
                nc.vector.tensor_copy(rg[:gsz], pt[:gsz])
                rgrps.append(rg)

            # mask rows on partition 0 for the PE broadcast (batched loads)
            mrows = []
            for mb in range(B // MROW_BATCH):
                mt = mrp.tile([1, MROW_BATCH, HW], F32, tag="mrow")
                nc.sync.dma_start(
                    mt, mask_d[mb * MROW_BATCH:(mb + 1) * MROW_BATCH, :]
                    .rearrange("(o s) f -> o s f", o=1))
                mrows.append(mt)

            # attended^T tiles, one per group: [128 c, KT, gsz]
            atts = [pp.tile([P, KT, gsz], F32R, tag=f"att{gi}", name=f"att{gi}")
                    for gi, (g0, gsz, e) in enumerate(groups)]
            if not do_pool:
                for att in atts:
                    nc.gpsimd.memset(att.bitcast(F32), 0.0)

            # ---- phase 1: masked pooling, one sample at a time ----
            sample_group = {}
            for gi, (g0, gsz, e) in enumerate(groups):
                for s in range(g0, g0 + gsz):
                    sample_group[s] = (gi, s - g0)
            for s in range(B):
                gi, pos = sample_group[s]
                ft = fp.tile([P, KT, HW], F32, tag="feat")
                nc.sync.dma_start(ft, feat_d[s].rearrange("(t p) f -> p t f", p=P))
                if do_pool:
                    bc = pbc.tile([P, HW], F32, name="bc")
                    if pool_mode in ("full", "bconly"):
                        nc.gpsimd.partition_broadcast(
                            bc, mrows[s // MROW_BATCH][0:1, s % MROW_BATCH])
                    if pool_mode == "full":
                        in1s = [bc] * KT
                    elif pool_mode == "sttsbuf":
                        in1s = [ft[:, t] for t in range(KT)]
                    else:
                        in1s = None
                    if in1s is not None:
                        for t in range(KT):
                            nc.vector.scalar_tensor_tensor(
                                dummy.broadcast_to([P, HW]),
                                ft[:, t], 1.0, in1s[t],
                                op0=mybir.AluOpType.mult, op1=mybir.AluOpType.mult,
                                accum_out=atts[gi][:, t, pos:pos + 1])

            # ---- phase 2: grouped GEMM over answer chunks ----
            bias_tiles = {}
            for gi, (g0, gsz, e) in enumerate(groups):
                if e not in bias_tiles:
                    bt = bp.tile([1, A], F32R, tag="bias")
                    nc.sync.dma_start(bt, bias_d[0:1, e * A:(e + 1) * A])
                    bias_tiles[e] = bt
                bt = bias_tiles[e]
                att = atts[gi]
                for (c0, cw) in CHUNKS:
                    wt = wtp.tile([P, KT, cw], F32R, tag="wt")
                    nc.sync.dma_start(
                        wt, wt_d[e].rearrange("(t p) a -> p t a", p=P)[:, :, c0:c0 + cw])
                    ot = op.tile([P, 512], F32, tag="out")
                    if not do_mm:
                        nc.gpsimd.memset(ot[:gsz, :cw], 0.0)
                    if do_mm:
                        ps = pmm.tile([P, 512], F32, name="ps")
                        for t in range(KT):
                            nc.tensor.matmul(
                                ps[:gsz, :cw],
                                lhsT=att[:, t],
                                rhs=wt[:, t],
                                start=(t == 0), stop=False)
                        nc.tensor.matmul(
                            ps[:gsz, :cw],
                            lhsT=msum_row[0:1, g0:g0 + gsz],
                            rhs=bt[0:1, c0:c0 + cw],
                            start=False, stop=True)
                        if do_evict:
                            nc.vector.tensor_scalar_mul(ot[:gsz, :cw], ps[:gsz, :cw],
                                                        rgrps[gi][:gsz])
                        else:
                            nc.gpsimd.memset(ot[:gsz, :cw], 0.0)
                    nc.sync.dma_start(part_d[g0:g0 + gsz, c0:c0 + cw],
                                      ot[:gsz, :cw])

    nc.compile()
    return nc


_PROGRAM_CACHE = {}


def _get_program(groups):
    key = tuple(groups)
    if key not in _PROGRAM_CACHE:
        _PROGRAM_CACHE[key] = build_program(groups)
    return _PROGRAM_CACHE[key]


def make_in_maps(mask, features, W, b, inst):
    """Host-side routing + sharding.  Returns (in_maps, perm, groups)."""
    inst_np = np.asarray(inst)
    perm = np.argsort(inst_np, kind="stable")
    counts = np.bincount(inst_np.astype(np.int64), minlength=E)
    groups = _make_groups(counts)

    mask_pad = np.ascontiguousarray(np.asarray(mask, np.float32).reshape(B, HW)[perm])

    feat = np.asarray(features, np.float32).reshape(B, C, HW)[perm]
    Wf = np.asarray(W, np.float32)
    bias_row = np.asarray(b, np.float32).reshape(1, E * A)
    zero_bias = np.zeros_like(bias_row)

    in_maps = []
    for k in range(NCORES):
        sl = slice(k * CS, (k + 1) * CS)
        feat_k = np.ascontiguousarray(feat[:, sl])
        wt_k = np.ascontiguousarray(Wf[:, :, sl].transpose(0, 2, 1))
        in_maps.append({
            "feat": feat_k,
            "mask": mask_pad,
            "wt": wt_k,
            "bias": bias_row if k == 0 else zero_bias,
        })
    return in_maps, perm, groups


def postprocess(results, perm):
    part = np.zeros((B, A), np.float32)
    for r in results:
        part += r["part"]
    out = np.empty((B, A), np.float32)
    out[perm] = part
    return out


def kernel(mask, features, W, b, inst):
    in_maps, perm, groups = make_in_maps(mask, features, W, b, inst)
    nc = _get_program(groups)
    res = bass_utils.run_bass_kernel_spmd(nc, in_maps, core_ids=list(range(NCORES)))
    return postprocess(res.results, perm)



# revision 8
# speedup vs baseline: 1.2819x; 1.2819x over previous
"""
MoE-routing kernel for Trainium2 (8 NeuronCores, SPMD via bass).

Computation (matches the reference):
  attended[b, c] = sum_hw(mask[b, hw] * feat[b, c, hw]) / sum_hw(mask[b, hw] + 1e-10)
  out[b, a]      = attended[b, :] @ W[inst[b], a, :] + bias[inst[b], a]

Strategy: split the channel dim C=2048 into 8 shards of 256 (one per core).
Each core computes a partial contraction over its channel shard for ALL 256
samples; the host sums the 8 partials.  The batch is sorted by expert on the
host (static routing baked into the compiled program).

All bulk data moves in bf16 with long contiguous per-partition DMA lines
(host repacks layouts).  Per core:

  phase 1 (pooling) on the PE: features are host-transposed to [hw, c] per
    sample (hw split 98+98 on partitions).  For each sample, 4 small matmuls
    (2 hw-chunks x 2 c-chunks) with the feature tile stationary and the
    sample's mask column moving produce one attended^T column [128c, 1] in
    PSUM, accumulated over hw-chunks.  No DVE work (DVE ops are 1x-mode and
    would bottleneck), no gpsimd broadcasts.
  phase 2 (grouped GEMM) on the PE: per expert group, stationary =
    attended^T columns (bf16), moving = W^T [c, a] chunks (bf16).  A K=1
    matmul accumulates msum[b] * bias[e, a] into PSUM; the ACT engine evicts
    PSUM with a per-partition scale of 1/msum[b] (normalizing the pooled
    features while leaving bias intact), writing bf16 partials.
"""

import sys

if "/opt/trn_rl_repo" not in sys.path:
    sys.path.insert(0, "/opt/trn_rl_repo")

import numpy as np

import concourse.bass as bass
import concourse.mybir as mybir
import concourse.tile as tile
from concourse import bacc
from concourse import bass_utils
from concourse.masks import make_identity

# Problem constants (hardcoded; kernel.py must be self-contained)
B = 256          # batch
C = 2048         # channels
HW = 196         # spatial positions (14*14)
E = 16           # experts
A = 3000         # answers
NCORES = 8
CS = C // NCORES  # channel shard per core = 256
P = 128
KT = CS // P      # c-tiles per core = 2
SB = 16           # samples per feature DMA block
NBLK = B // SB    # 16 feature blocks
HC = 98           # hw chunk (196 = 2*98), partition dim of feat tiles
NH = 2
ACH = [(c0, 500) for c0 in range(0, A, 500)]  # GEMM answer chunks (moving <= 512)

F32 = mybir.dt.float32
BF16 = mybir.dt.bfloat16
AF = mybir.ActivationFunctionType


def _make_groups(counts):
    """[(gstart_in_sorted_order, gsz, expert)] with gsz <= 128."""
    groups = []
    start = 0
    for e in range(E):
        n = int(counts[e])
        g0 = start
        while n > 0:
            gsz = min(n, P)
            groups.append((g0, gsz, e))
            g0 += gsz
            n -= gsz
        start += int(counts[e])
    return groups


def build_program(groups, loop_n=1, do_pool=True, do_mm=True, do_evict=True):
    """Build + compile the per-core Bass program (identical on all cores)."""
    nc = bacc.Bacc("TRN2", target_bir_lowering=False, debug=False,
                   num_devices=NCORES)

    featT_d = nc.dram_tensor("featT", [NBLK * HC, SB * NH * CS], BF16,
                             kind="ExternalInput").ap()
    # mask columns duplicated x2: bf16 matmuls need >=2 moving elements
    mcol_d = nc.dram_tensor("mcol", [HC, NH * B * 2], BF16,
                            kind="ExternalInput").ap()
    maskf_d = nc.dram_tensor("maskf", [B, HW], F32, kind="ExternalInput").ap()
    wt_d = nc.dram_tensor("wt", [E, P, KT * A], BF16,
                          kind="ExternalInput").ap()
    bias_d = nc.dram_tensor("bias", [1, E * A], BF16,
                            kind="ExternalInput").ap()
    part_d = nc.dram_tensor("part", [B, A], BF16, kind="ExternalOutput").ap()

    import contextlib
    with tile.TileContext(nc) as tc:
        loop_ctx = tc.For_i(0, loop_n, 1) if loop_n > 1 else contextlib.nullcontext()
        with (
            loop_ctx,
            tc.tile_pool(name="persist", bufs=1) as pp,
            tc.tile_pool(name="feat", bufs=3) as fp,
            tc.tile_pool(name="wt", bufs=3) as wtp,
            tc.tile_pool(name="bias", bufs=2) as bp,
            tc.tile_pool(name="outs", bufs=3) as op,
            tc.tile_pool(name="att", bufs=4) as abp,
            tc.tile_pool(name="ps_a", bufs=4, space="PSUM") as pa,
            tc.tile_pool(name="ps_g", bufs=2, space="PSUM") as pg,
        ):
            # ---- constants ----
            ident = pp.tile([P, P], F32, tag="ident")
            make_identity(nc, ident)
            ones32 = pp.tile([1, 1], F32, tag="ones32")
            nc.vector.memset(ones32, 1.0)

            # ---- mask: per-sample sums, reciprocals, bf16 row ----
            mbp = pp.tile([P, KT, HW], F32, tag="mbp")
            nc.sync.dma_start(mbp, maskf_d.rearrange("(t p) f -> p t f", p=P))
            msum = pp.tile([P, KT], F32, tag="msum")
            nc.vector.tensor_reduce(msum, mbp,
                                    axis=mybir.AxisListType.X,
                                    op=mybir.AluOpType.add)
            nc.vector.tensor_scalar_add(msum, msum, HW * 1e-10)

            msum_row = pp.tile([1, B], F32, tag="msum_row")
            for t in range(KT):
                pt = pa.tile([1, P], F32, name="pt_row", tag="pa")
                nc.tensor.matmul(pt, lhsT=msum[:, t:t + 1], rhs=ident,
                                 start=True, stop=True)
                nc.vector.tensor_copy(msum_row[0:1, t * P:(t + 1) * P], pt)
            recip_row = pp.tile([1, B], F32, tag="recip_row")
            nc.vector.reciprocal(recip_row, msum_row)
            msum_row_bf = pp.tile([1, B], BF16, tag="msum_row_bf")
            nc.vector.tensor_copy(msum_row_bf, msum_row)

            # per-group reciprocals at partition base 0: rg[r] = 1/msum[g0+r]
            rgrps = []
            for gi, (g0, gsz, e) in enumerate(groups):
                rg = pp.tile([P, 1], F32, tag=f"rgrp{gi}", name=f"rgrp{gi}")
                ptc = pa.tile([P, 1], F32, name="pt_col", tag="pa")
                nc.tensor.matmul(ptc[:gsz], lhsT=recip_row[0:1, g0:g0 + gsz],
                                 rhs=ones32[0:1, 0:1], start=True, stop=True)
                nc.vector.tensor_copy(rg[:gsz], ptc[:gsz])
                rgrps.append(rg)

            # mask columns for the pooling matmuls: [98, (h*B + s)*2 + dup]
            mct = pp.tile([HC, NH * B * 2], BF16, tag="mct")
            nc.sync.dma_start(mct, mcol_d)

            # ---- phases: pooling (PE) + grouped GEMM (PE) per group ----
            bias_tiles = {}
            ft = None
            cur_blk = -1
            for gi, (g0, gsz, e) in enumerate(groups):
                # pooling: one attended^T column per sample, 2 c-chunks
                aps = [pa.tile([P, gsz, 2], F32, name=f"att_ps{cc}", tag="pa")
                       for cc in range(KT)]
                for pos in range(gsz):
                    s = g0 + pos
                    blk, j = divmod(s, SB)
                    if blk != cur_blk:
                        ft = fp.tile([HC, SB * NH * CS], BF16, tag="feat")
                        nc.sync.dma_start(
                            ft, featT_d[blk * HC:(blk + 1) * HC, :])
                        cur_blk = blk
                    if do_pool:
                        for cc in range(KT):
                            for h in range(NH):
                                nc.tensor.matmul(
                                    aps[cc][:, pos, :],
                                    lhsT=ft[:, (j * NH + h) * CS + cc * P:
                                            (j * NH + h) * CS + (cc + 1) * P],
                                    rhs=mct[:, (h * B + s) * 2:(h * B + s) * 2 + 2],
                                    start=(h == 0), stop=(h == NH - 1))
                att_sb = abp.tile([P, KT * gsz], BF16, tag="att")
                for cc in range(KT):
                    if do_pool:
                        nc.scalar.copy(att_sb[:, cc * gsz:(cc + 1) * gsz],
                                       aps[cc][:, :, 0])
                    else:
                        nc.gpsimd.memset(att_sb[:, cc * gsz:(cc + 1) * gsz], 0.0)

                # grouped GEMM for this expert
                if e not in bias_tiles:
                    bt = bp.tile([1, A], BF16, tag="bias")
                    nc.sync.dma_start(bt, bias_d[0:1, e * A:(e + 1) * A])
                    bias_tiles[e] = bt
                bt = bias_tiles[e]
                wt = wtp.tile([P, KT * A], BF16, tag="wt")
                nc.sync.dma_start(wt, wt_d[e])
                ot = op.tile([P, A], BF16, tag="out")
                for (c0, cw) in ACH:
                    if do_mm:
                        ps = pg.tile([P, 500], F32, name="ps", tag="pg")
                        for t in range(KT):
                            nc.tensor.matmul(
                                ps[:gsz, :cw],
                                lhsT=att_sb[:, t * gsz:(t + 1) * gsz],
                                rhs=wt[:, t * A + c0:t * A + c0 + cw],
                                start=(t == 0), stop=False)
                        nc.tensor.matmul(
                            ps[:gsz, :cw],
                            lhsT=msum_row_bf[0:1, g0:g0 + gsz],
                            rhs=bt[0:1, c0:c0 + cw],
                            start=False, stop=True)
                        if do_evict:
                            nc.scalar.mul(ot[:gsz, c0:c0 + cw], ps[:gsz, :cw],
                                          rgrps[gi][:gsz])
                        else:
                            nc.gpsimd.memset(ot[:gsz, c0:c0 + cw], 0.0)
                    else:
                        nc.gpsimd.memset(ot[:gsz, c0:c0 + cw], 0.0)
                nc.sync.dma_start(part_d[g0:g0 + gsz, :], ot[:gsz, :])

    nc.compile()
    return nc


_PROGRAM_CACHE = {}


def _get_program(groups):
    key = tuple(groups)
    if key not in _PROGRAM_CACHE:
        _PROGRAM_CACHE[key] = build_program(groups)
    return _PROGRAM_CACHE[key]


def make_in_maps(mask, features, W, b, inst):
    """Host-side routing + sharding + bf16 repack.  Returns (in_maps, perm, groups)."""
    np_bf16 = mybir.dt.np(BF16)
    inst_np = np.asarray(inst)
    perm = np.argsort(inst_np, kind="stable")
    counts = np.bincount(inst_np.astype(np.int64), minlength=E)
    groups = _make_groups(counts)

    mask_p = np.asarray(mask, np.float32).reshape(B, HW)[perm]
    mask_f32 = np.ascontiguousarray(mask_p)
    # mask columns [98, (h*B + s)*2 + dup] (hw = h*98 + p), bf16, duplicated
    mcol = (mask_p.astype(np_bf16).T.reshape(NH, HC, B).transpose(1, 0, 2)
            .reshape(HC, NH * B))
    mcol = np.ascontiguousarray(np.repeat(mcol, 2, axis=1))

    feat = np.asarray(features, np.float32).reshape(B, C, HW)[perm]
    Wf = np.asarray(W, np.float32)
    bias_row = np.asarray(b, np.float32).astype(np_bf16).reshape(1, E * A)
    zero_bias = np.zeros_like(bias_row)

    in_maps = []
    for k in range(NCORES):
        sl = slice(k * CS, (k + 1) * CS)
        # features: [B, CS, HW] -> [NBLK*98, (j, h, c)] transposed to hw-major
        fk = feat[:, sl].transpose(0, 2, 1).astype(np_bf16)       # [B, HW, CS]
        fk = fk.reshape(NBLK, SB, NH, HC, CS).transpose(0, 3, 1, 2, 4)
        fk = np.ascontiguousarray(fk.reshape(NBLK * HC, SB * NH * CS))
        # W: [E, A, CS] -> [E, 128, (t, a)]
        wk = Wf[:, :, sl].transpose(0, 2, 1).astype(np_bf16)      # [E, CS, A]
        wk = wk.reshape(E, KT, P, A).transpose(0, 2, 1, 3)
        wk = np.ascontiguousarray(wk.reshape(E, P, KT * A))
        in_maps.append({
            "featT": fk,
            "mcol": mcol,
            "maskf": mask_f32,
            "wt": wk,
            "bias": bias_row if k == 0 else zero_bias,
        })
    return in_maps, perm, groups


def postprocess(results, perm):
    part = np.zeros((B, A), np.float32)
    for r in results:
        part += r["part"].astype(np.float32)
    out = np.empty((B, A), np.float32)
    out[perm] = part
    return out


def kernel(mask, features, W, b, inst):
    in_maps, perm, groups = make_in_maps(mask, features, W, b, inst)
    nc = _get_program(groups)
    res = bass_utils.run_bass_kernel_spmd(nc, in_maps, core_ids=list(range(NCORES)))
    return postprocess(res.results, perm)
